# revision 44
# baseline (speedup 1.0000x reference)
"""Self-contained Trainium2 Bass kernel for nn_CA_9363028705415 (sparse_attention).

Computes, per batch b:
    Q = relu(x[b] @ qW1) @ qW2                      # [M, K]   (biases are 0)
    Kt = relu(x[b] @ kW1) @ kW2                     # [M, K]
    S = Q @ Kt.T                                    # [M, M]
    out[b] = softmax(S / rowmax(S), axis=-1)        # max-DIVISION normalization

Shapes: B=16, M=2048, D=128, H=256, K=64.  Output [16, 2048, 2048] returned as
f32; the device writes fp16 (rel tolerance 2e-2 dwarfs fp16 quantization) so
the HBM write is 16 MB/core instead of 32 MB (DMA roofline 358 GB/s/core).

Sharding: data-parallel over batch across 8 NeuronCores; 2 batches/core; tiny
MLP weights replicated.  Single NEFF run SPMD via run_bass_kernel_spmd.

Per 128-row tile (engines balanced so DVE and ACT both run ~3.1us/tile):
  PE:  S = Q K^T into a 4-bank f32 PSUM tile (4 x N=512 bf16 matmuls)
  DVE: fused PSUM->SBUF fp16 evac + row-max (tensor_scalar accum max) —
       frees the PSUM slot so the next tile's matmuls don't wait on ACT
  DVE: reciprocal of [max(rt) | sum(rt-1)]            (tiny)
  ACT: t = exp(sc * (1/max)) from the fp16 SBUF copy, fused row-sum
       accumulator -> pairs[rt+1][:,1]
  DVE: out = t * (1/sum), fp16 -> fp16 runs in 4x mode (0.6us)
  HWDGE DMA: 1 MB fp16 output chunks (2 row-tiles)
All phase-A evacs (relu / f32->bf16 casts, transpose evac) run on ACT so the
DVE steady state is exactly evac+max / recip / norm.  The next batch's
MLP/transpose work is interleaved into the current batch's S loop in chunks.
"""

import numpy as np
import ml_dtypes

import concourse.bass as bass
import concourse.mybir as mybir
from concourse import bacc
import concourse.tile as tile
from concourse.bass import ts
from concourse.bass_utils import run_bass_kernel_spmd

F32 = mybir.dt.float32
BF16 = mybir.dt.bfloat16
FP16 = mybir.dt.float16
AF = mybir.ActivationFunctionType
ALU = mybir.AluOpType
AXX = mybir.AxisListType.X

N_CORES = 8
B, M, D, H, KF = 16, 2048, 128, 256, 64
BPC = B // N_CORES     # batches per core
MT = M // 128          # 16 row-tiles per batch
FC = M // 512          # 4 matmul free-chunks of 512
PAIR = 2               # row-tiles per output DMA (1 MB fp16 chunks)


def build_nc():
    nc = bacc.Bacc()

    x = nc.dram_tensor("x", [BPC, M, D], F32, kind="ExternalInput")
    w1d, w2d = {}, {}
    for h in ("q", "k"):
        w1d[h] = nc.dram_tensor(f"{h}W1", [D, H], F32, kind="ExternalInput")
        w2d[h] = nc.dram_tensor(f"{h}W2", [H, KF], F32, kind="ExternalInput")
    out = nc.dram_tensor("out", [BPC, M, M], FP16, kind="ExternalOutput")

    ident_f32_np = np.eye(128, dtype=np.float32)
    ident_f32_dram = nc.inline_tensor(ident_f32_np, name="ident_f32_data")

    # [b, p, n, d]: token (n*128+p), feature d
    x_r = x[:].rearrange("b (n p) d -> b p n d", p=128)
    # [b, p, n, m]: out[b, n*128+p, m]
    out_r = out[:].rearrange("b (n p) m -> b p n m", p=128)

    with tile.TileContext(nc) as tc:
        with (
            tc.tile_pool(name="consts", bufs=1) as consts,
            tc.tile_pool(name="xin", bufs=2) as xin_pool,
            tc.tile_pool(name="xt", bufs=2) as xt_pool,
            tc.tile_pool(name="ht", bufs=2) as ht_pool,
            tc.tile_pool(name="qkt", bufs=2) as qkt_pool,
            tc.tile_pool(name="texp", bufs=3) as t_pool,
            tc.tile_pool(name="osb", bufs=3) as out_pool,
            tc.tile_pool(name="small", bufs=8) as small_pool,
            tc.tile_pool(name="psum", bufs=2, space="PSUM") as psum_pool,
        ):
            # ---- x load for batch 0 first: nothing can start without x ----
            xf = {}
            for b in range(BPC):
                xf[b] = xin_pool.tile([128, MT, 128], F32, tag=f"xf{b}", name="xf")
            # two HWDGE rings (sync + scalar) drain the slow 512B-descriptor
            # x halves in parallel instead of FIFO-serial on one queue
            nc.sync.dma_start(out=xf[0][:, 0:8, :], in_=x_r[0][:, 0:8, :])
            nc.scalar.dma_start(out=xf[0][:, 8:16, :], in_=x_r[0][:, 8:16, :])

            # ---- PE warm-up: ~4.5us of junk matmuls on a memset tile while
            # the x DMA is in flight releases the HAM clock gate (cold PE
            # runs at 1.2 GHz; sustained activity unlocks 2.4 GHz) so the
            # phase-A matmuls and early S tiles run warm.
            junk_in = consts.tile([128, 512], BF16, tag="junk", name="junk_in")
            nc.vector.memset(junk_in, 0.0)
            for jp in range(2):
                ps_j = psum_pool.tile([128, 512], F32, tag="ps", name="ps_j")
                for _ in range(6):
                    nc.tensor.matmul(
                        ps_j, lhsT=junk_in[:, 0:128], rhs=junk_in,
                        start=True, stop=True,
                    )

            # ---- constants (identity inline; weights cast f32->bf16 by DMA) --
            ident_f32 = consts.tile([128, 128], F32, tag="ident")
            nc.sync.dma_start(out=ident_f32, in_=ident_f32_dram[:])
            w1, w2 = {}, {}
            for h in ("q", "k"):
                w1[h] = consts.tile([D, H], BF16, tag=f"w1{h}", name=f"w1{h}")
                nc.gpsimd.dma_start(out=w1[h], in_=w1d[h][:])  # cast f32->bf16
                w2[h] = consts.tile([128, 2, KF], BF16, tag=f"w2{h}", name=f"w2{h}")
                nc.gpsimd.dma_start(
                    out=w2[h], in_=w2d[h][:].rearrange("(c p) k -> p c k", p=128)
                )
            for g in range(2):
                nc.sync.dma_start(
                    out=xf[1][:, g * 8 : (g + 1) * 8, :],
                    in_=x_r[1][:, g * 8 : (g + 1) * 8, :],
                )

            # preload the ACT exp table set during the initial DMA wait so the
            # first real exp doesn't pay the ~2.7us ACT_TABLE_LOAD mid-loop
            warm = small_pool.tile([128, 2], F32, tag="warm", name="warm")
            nc.vector.memset(warm[:, 0:1], 0.0)
            nc.scalar.activation(warm[:, 1:2], warm[:, 0:1], AF.Exp)

            def phase_a_chunks(b, fast=False):
                """Phase-A emission chunks for batch b, fine-grained so the
                serial ramp is short and chunks interleave into the previous
                batch's S loop.  DVE: cast + transpose evac; ACT: mlp evacs."""
                ctx = {}

                def c_tp(g, eng):
                    # f32 transpose straight from the DMA'd x tile; the evac
                    # casts f32 PSUM -> bf16 xT (same engine cost as a plain
                    # copy, so the separate x cast pass disappears)
                    def go():
                        if "xT" not in ctx:
                            ctx["xT"] = xt_pool.tile(
                                [128, M], BF16, tag="xt", name="xT"
                            )
                        for half in range(2):
                            tp = psum_pool.tile([128, 512], F32, tag="ps", name="tp")
                            for it in range(4):
                                nc.tensor.transpose(
                                    tp[:, ts(it, 128)],
                                    xf[b][:, g * 8 + half * 4 + it, :],
                                    ident_f32,
                                )
                            dst = ctx["xT"][:, g * 1024 + half * 512 : g * 1024 + (half + 1) * 512]
                            if eng == "act":
                                nc.scalar.copy(dst, tp)
                            else:
                                nc.vector.tensor_copy(dst, tp)
                    return go

                def c_mlp1(h, pc, half, eng):
                    def go():
                        if ("ht", h) not in ctx:
                            ctx[("ht", h)] = ht_pool.tile(
                                [128, 2, M], BF16, tag=f"ht{h}", name=f"ht{h}"
                            )
                        ps1 = psum_pool.tile([128, 1024], F32, tag="ps", name="ps1")
                        for fc in range(2):
                            nc.tensor.matmul(
                                ps1[:, ts(fc, 512)],
                                lhsT=w1[h][:, ts(pc, 128)],
                                rhs=ctx["xT"][:, ts(half * 2 + fc, 512)],
                                start=True,
                                stop=True,
                            )
                        dst = ctx[("ht", h)][:, pc, ts(half, 1024)]
                        if eng == "act":
                            nc.scalar.activation(dst, ps1, AF.Relu)
                        else:
                            nc.vector.tensor_scalar(dst, ps1, 0.0, None, op0=ALU.max)
                    return go

                def c_mlp2_mm(h):
                    def go():
                        q = qkt_pool.tile([KF, M], BF16, tag=f"qkt{h}", name=f"qkt{h}")
                        ctx[("qkt", h)] = q
                        ps2 = psum_pool.tile([KF, M], F32, tag="ps", name="ps2")
                        ctx[("ps2", h)] = ps2
                        for fc in range(FC):
                            for kc in range(2):
                                nc.tensor.matmul(
                                    ps2[:, ts(fc, 512)],
                                    lhsT=w2[h][:, kc, :],
                                    rhs=ctx[("ht", h)][:, kc, ts(fc, 512)],
                                    start=(kc == 0),
                                    stop=(kc == 1),
                                )
                    return go

                def c_mlp2_evac(h, fc0, fc1, eng):
                    def go():
                        q = ctx[("qkt", h)]
                        ps2 = ctx[("ps2", h)]
                        for fc in range(fc0, fc1):
                            src = ps2[:, ts(fc, 512)]
                            dst = q[:, ts(fc, 512)]
                            if eng == "act":
                                nc.scalar.copy(dst, src)
                            else:
                                nc.vector.tensor_copy(dst, src)
                    return go

                def c_mlp2_steady(h):
                    def go():
                        c_mlp2_mm(h)()
                        nc.scalar.copy(ctx[("qkt", h)], ctx[("ps2", h)])
                    return go

                if fast:
                    # batch 0 ramp: split evacs across both engines and
                    # interleave q/k mlp2 evac chunks so the first S matmuls
                    # (needing qT tile 0 + kT chunk 0) start as early as
                    # possible
                    chunks = [c_tp(0, "dve"), c_tp(1, "act")]
                    for i, (h, pc, half) in enumerate(
                        [
                            ("q", 0, 0), ("k", 0, 0), ("q", 0, 1), ("k", 0, 1),
                            ("q", 1, 0), ("k", 1, 0), ("q", 1, 1), ("k", 1, 1),
                        ]
                    ):
                        chunks.append(c_mlp1(h, pc, half, ("act", "dve")[i % 2]))
                    chunks.append(c_mlp2_mm("q"))
                    chunks.append(c_mlp2_mm("k"))
                    chunks.append(c_mlp2_evac("q", 0, 1, "act"))
                    chunks.append(c_mlp2_evac("k", 0, 1, "dve"))
                    chunks.append(c_mlp2_evac("q", 1, 2, "act"))
                    chunks.append(c_mlp2_evac("k", 1, 2, "dve"))
                    chunks.append(c_mlp2_evac("q", 2, 4, "act"))
                    chunks.append(c_mlp2_evac("k", 2, 4, "dve"))
                else:
                    chunks = [c_tp(0, "act"), c_tp(1, "act")]
                    for h, pc, half in [
                        ("q", 0, 0), ("k", 0, 0), ("q", 0, 1), ("k", 0, 1),
                        ("q", 1, 0), ("k", 1, 0), ("q", 1, 1), ("k", 1, 1),
                    ]:
                        chunks.append(c_mlp1(h, pc, half, "act"))
                    chunks.append(c_mlp2_steady("q"))
                    chunks.append(c_mlp2_steady("k"))
                return ctx, chunks

            def s_loop(b, qkt, next_chunks):
                """S + softmax loop for batch b, interleaving next_chunks
                (next batch's phase A) into its iterations."""
                osb_tiles = {}
                pending = None

                def finish(j, t_j, isum_ap):
                    if j >= MT - 2:
                        # tail: single-tile staging + immediate DMA so the
                        # last transfers overlap the final exp/norm chain
                        osb = out_pool.tile([128, M], FP16, tag="o1", name="osb1")
                        nc.vector.tensor_scalar_mul(osb, t_j, isum_ap)
                        nc.sync.dma_start(out=out_r[b][:, j : j + 1, :], in_=osb)
                        return
                    nc.vector.tensor_scalar_mul(
                        osb_tiles[j // PAIR][:, ts(j % PAIR, M)], t_j, isum_ap
                    )
                    if j % PAIR == PAIR - 1:
                        osb = osb_tiles.pop(j // PAIR)
                        nc.sync.dma_start(
                            out=out_r[b][:, j - PAIR + 1 : j + 1, :],
                            in_=osb,
                        )

                # pairs[rt] holds [row-max(rt) | exp-row-sum(rt-1)]; one
                # reciprocal per tile covers both 1/max(rt) and 1/sum(rt-1).
                pairs = {0: small_pool.tile([128, 2], F32, tag="pr", name="pair")}
                nc.vector.memset(pairs[0], 1.0)
                for rt in range(MT):
                    ps_s = psum_pool.tile([128, M], F32, tag="ps", name="ps_s")
                    for fc in range(FC):
                        nc.tensor.matmul(
                            ps_s[:, ts(fc, 512)],
                            lhsT=qkt["q"][:, ts(rt, 128)],
                            rhs=qkt["k"][:, ts(fc, 512)],
                            start=True,
                            stop=True,
                        )
                    # Fused PSUM->SBUF fp16 evac + row-max; frees the PSUM
                    # slot so the next tile's matmuls only wait on this DVE op
                    sc_t = t_pool.tile([128, M], FP16, tag="sc", name="sc")
                    nc.vector.tensor_scalar(
                        sc_t,
                        ps_s,
                        0.0,
                        None,
                        op0=ALU.add,
                        op1=ALU.max,
                        accum_out=pairs[rt][:, 0:1],
                    )

                    ipair = small_pool.tile([128, 2], F32, tag="ip", name="ipair")
                    nc.vector.reciprocal(ipair, pairs[rt])
                    pairs[rt + 1] = small_pool.tile([128, 2], F32, tag="pr", name="pair")

                    # exp from the fp16 SBUF copy; fused row-sum accumulator
                    t_t = t_pool.tile([128, M], FP16, tag="t")
                    nc.scalar.activation(
                        t_t,
                        sc_t,
                        AF.Exp,
                        bias=0.0,
                        scale=ipair[:, 0:1],
                        accum_out=pairs[rt + 1][:, 1:2],
                    )

                    if rt % PAIR == 0 and rt < MT - 2:
                        osb_tiles[rt // PAIR] = out_pool.tile(
                            [128, PAIR * M], FP16, tag="o", name="osb"
                        )
                    if pending is not None:
                        finish(pending[0], pending[1], ipair[:, 1:2])
                    pending = (rt, t_t)

                    # interleave the next batch's MLP work
                    if next_chunks:
                        next_chunks.pop(0)()
                last_is = small_pool.tile([128, 1], F32, tag="li", name="last_is")
                nc.vector.reciprocal(last_is, pairs[MT][:, 1:2])
                finish(pending[0], pending[1], last_is)
                while next_chunks:
                    next_chunks.pop(0)()

            ctx0, chunks0 = phase_a_chunks(0, fast=True)
            for c in chunks0:
                c()
            qkt0 = {"q": ctx0[("qkt", "q")], "k": ctx0[("qkt", "k")]}

            ctx1, chunks1 = phase_a_chunks(1)
            s_loop(0, qkt0, chunks1)
            qkt1 = {"q": ctx1[("qkt", "q")], "k": ctx1[("qkt", "k")]}
            s_loop(1, qkt1, [])
    nc.finalize()
    _strip_dup_ldweights(nc)
    return nc


def _strip_dup_ldweights(nc):
    """Drop back-to-back duplicate InstLdweights (same weights AP, only
    matmuls between) from the scheduled module.  bass pairs every
    non-transpose matmul with its own InstLdweights even when consecutive
    matmuls share the stationary operand (4 S-matmuls per row-tile reload
    one qT slice); the PE keeps weights across matmuls, so the duplicates
    are pure queue overhead (~147 ns x ~80 per core).  Only sync-free,
    dependency-unreferenced duplicates are removed (verified: all 80)."""
    refs = set()
    for f in nc.m.functions:
        for blk in f.blocks:
            for i in blk.instructions:
                try:
                    refs.update(i.sync_dependency_names())
                    refs.update(i.nosync_dependency_names())
                except Exception:
                    pass
    for f in nc.m.functions:
        for blk in f.blocks:
            il = blk.instructions
            keep = []
            prev_w = None
            for i in il:
                if isinstance(i, mybir.InstLdweights):
                    w = str(i.ins[0]) if i.ins else None
                    if (
                        w is not None
                        and w == prev_w
                        and not i.has_wait()
                        and not i.has_update()
                        and i.name not in refs
                    ):
                        continue  # drop duplicate
                    prev_w = w
                elif isinstance(i, mybir.InstMatmult):
                    pass  # matmuls don't disturb the loaded weights
                else:
                    prev_w = None
                keep.append(i)
            if len(keep) != len(il):
                il[:] = keep


_NC_CACHE = None


def _get_nc():
    global _NC_CACHE
    if _NC_CACHE is None:
        _NC_CACHE = build_nc()
    return _NC_CACHE


def run(inputs, trace=False, trace_cores=None):
    """Run on 8 cores; returns (full_output [B,M,M] f32, BassKernelResults)."""
    nc = _get_nc()
    in_maps = []
    x = np.ascontiguousarray(inputs["x"], dtype=np.float32)
    for c in range(N_CORES):
        im = {"x": np.ascontiguousarray(x[c * BPC : (c + 1) * BPC])}
        for k in ("qW1", "qW2", "kW1", "kW2"):
            im[k] = np.ascontiguousarray(inputs[k], dtype=np.float32)
        in_maps.append(im)
    res = run_bass_kernel_spmd(
        nc,
        in_maps,
        core_ids=list(range(N_CORES)),
        trace=trace,
        trace_cores=trace_cores,
    )
    outs = [r["out"] for r in res.results]
    full = np.concatenate(outs, axis=0).astype(np.float32)
    assert full.shape == (B, M, M) and full.dtype == np.float32
    return full, res


def kernel(**inputs) -> np.ndarray:
    out, _ = run(inputs, trace=False)
    return out
